# revision 67
# baseline (speedup 1.0000x reference)
"""Varlen causal GQA attention on 8 TRN2 NeuronCores.

Problem: 32 q heads, 8 kv heads, head_dim 128, ragged batch (cu_seqlens),
f32. Sharded by KV-head group: core c owns kv head c and q heads
4c..4c+3 — fully data-independent across cores, no collectives.

Per core, blockwise causal attention in 128x128 blocks with all 4 q
heads fused through 3D access patterns (q stored head-interleaved
[d, h, t]), so every matmul streams exactly 4*128 = 512 columns and
each PSUM bank carries exactly one accumulation chain:
    S[k, h, q]  = (K_j)^T.T @ Q^T      ONE matmul per (q-block, k-block)
    P = exp(S * scale)                 ONE ScalarE op
    causal mask: 0/1 multiply on GpSimd (diagonal blocks only)
    O^T[h] += V_j @ P                  ONE matmul, PSUM-accumulated over j
    sums[h] += ones.T @ P              ONE M=1 matmul
Host does all transposes (Q^T/K^T in, O^T -> O out), bf16 conversion,
and the final softmax division.
"""

import math
import os
import sys

sys.path.insert(0, "/opt/trn_rl_repo")

import ml_dtypes
import numpy as np

NUM_HEADS = 32
NUM_KV_HEADS = 8
HEAD_DIM = 128
HEADS_PER_CORE = NUM_HEADS // NUM_KV_HEADS  # 4
N_CORES = 8
BLK = 128
SCALE = 1.0 / math.sqrt(HEAD_DIM)

_GRAPH_CACHE = {}


def _build_graph(seq_blocks):
    """Build the SPMD Bacc graph for padded per-seq block counts."""
    from concourse import bacc
    import concourse.mybir as mybir
    from concourse.tile import TileContext

    f32 = mybir.dt.float32
    bf16 = mybir.dt.bfloat16
    T = sum(seq_blocks) * BLK
    n_blocks_total = T // BLK
    H = HEADS_PER_CORE

    nc = bacc.Bacc("TRN2", target_bir_lowering=False, debug=False,
                   num_devices=N_CORES)

    qT_ext = nc.declare_dram_parameter("qT", [BLK, H, T], bf16, isOutput=False)
    kT_ext = nc.declare_dram_parameter("kT", [BLK, T], bf16, isOutput=False)
    v_ext = nc.declare_dram_parameter("v", [T, HEAD_DIM], bf16, isOutput=False)
    mask_ext = nc.declare_dram_parameter("mask", [BLK, H, BLK], bf16,
                                         isOutput=False)
    oT_ext = nc.declare_dram_parameter("oT", [BLK, H, T], f32, isOutput=True)
    sums_ext = nc.declare_dram_parameter("sums", [1, H, T], f32, isOutput=True)

    with TileContext(nc) as tc:
        with (
            tc.tile_pool(name="persist", bufs=1) as persist,
            tc.tile_pool(name="p", bufs=8) as p_pool,
            tc.tile_pool(name="ps_s4", bufs=2, space="PSUM") as ps_s4,
            tc.tile_pool(name="ps_o", bufs=2, space="PSUM") as ps_o,
            tc.tile_pool(name="ps_sum", bufs=2, space="PSUM") as ps_sum,
        ):
            kT_sb = persist.tile([BLK, T], bf16)
            v_sb = persist.tile([BLK, n_blocks_total, HEAD_DIM], bf16)
            mask_sb = persist.tile([BLK, H, BLK], bf16)
            qT_sb = persist.tile([BLK, H, T], bf16)
            v_re = v_ext[:].rearrange("(j p) d -> p j d", p=BLK)
            nb0 = seq_blocks[0]
            # dependencies are tile-granular: duplicate the first few blocks
            # into separate small tiles so the opening matmuls depend only on
            # tiny DMAs, not on the full-tensor loads running behind them
            nh = min(6, nb0)
            c00 = nh * BLK
            kT_head = persist.tile([BLK, c00], bf16)
            qT_head = persist.tile([BLK, H, c00], bf16)
            v_head = persist.tile([BLK, nh, HEAD_DIM], bf16)
            # one ring, strict order: small head tiles land first (the ring
            # serializes transfers), then seq-0 bulk; other seqs + mask go on
            # the Scalar ring where they can't delay the heads
            nc.sync.dma_start(qT_head[:], qT_ext[:, :, :c00])
            nc.sync.dma_start(kT_head[:], kT_ext[:, :c00])
            nc.sync.dma_start(v_head[:], v_re[:, :nh, :])
            nc.sync.dma_start(qT_sb[:, :, : nb0 * BLK],
                              qT_ext[:, :, : nb0 * BLK])
            nc.sync.dma_start(kT_sb[:, : nb0 * BLK], kT_ext[:, : nb0 * BLK])
            nc.sync.dma_start(v_sb[:, :nb0, :], v_re[:, :nb0, :])
            nc.scalar.dma_start(mask_sb[:], mask_ext[:])
            if nb0 < n_blocks_total:
                c0 = nb0 * BLK
                nc.scalar.dma_start(kT_sb[:, c0:], kT_ext[:, c0:])
                nc.scalar.dma_start(v_sb[:, nb0:, :], v_re[:, nb0:, :])
                nc.scalar.dma_start(qT_sb[:, :, c0:], qT_ext[:, :, c0:])

            ones_f = persist.tile([BLK, BLK], f32)
            nc.vector.memset(ones_f[:], 1.0)
            # full [128,128] ones stationary: sums matmul runs M=128 so the
            # PE array never reconfigures col groups between AV and sums
            ones_b = persist.tile([BLK, BLK], bf16)
            nc.vector.tensor_copy(ones_b[:], ones_f[:])

            ot_stage = persist.tile([BLK, H, T], f32)
            sums_stage = persist.tile([1, H, T], f32)

            # flat chunk stream over (seq, q-block g, k-block j-pairs), with
            # j descending inside each group (diagonal/masked block first).
            # Each chunk = up to 2 consecutive j's whose S tiles share one
            # 2-bank PSUM tile [128, jj, h, q] so ONE exp covers both.
            chunks = []
            seq_off = 0
            for nblk in seq_blocks:
                for g in range(nblk):
                    js = list(range(g, -1, -1))
                    for i0 in range(0, len(js), 2):
                        chunks.append((seq_off, nblk, g, js[i0 : i0 + 2]))
                seq_off += nblk * BLK

            # software-pipelined ISSUE order: AV/sums trail their S/exp by
            # LAG chunks so the PE's FIFO never parks on an exp wait
            LAG = 2
            state = {}  # live group accumulators keyed by (seq_off, g)
            pending = []

            def emit_front(ch):
                seq_off, nblk, g, js = ch
                Q0 = seq_off + g * BLK
                s2 = ps_s4.tile([BLK, 2, H, BLK], f32, tag="s2", name="s2")
                for jj, j in enumerate(js):
                    if seq_off == 0 and g < nh:
                        kj = kT_head[:, j * BLK : (j + 1) * BLK]
                        qg = qT_head[:, :, g * BLK : (g + 1) * BLK]
                    else:
                        kj = kT_sb[:, seq_off + j * BLK : seq_off + (j + 1) * BLK]
                        qg = qT_sb[:, :, Q0 : Q0 + BLK]
                    nc.tensor.matmul(
                        s2[:, jj], kj, qg,
                        start=True, stop=True,
                    )
                p2 = p_pool.tile([BLK, 2, H, BLK], bf16, tag="p2", name="p2")
                nj = len(js)
                nc.scalar.activation(
                    p2[:, :nj], s2[:, :nj],
                    mybir.ActivationFunctionType.Exp,
                    scale=SCALE,
                )
                if js[0] == g:  # diagonal: zero the upper triangle
                    nc.gpsimd.tensor_mul(p2[:, 0], p2[:, 0], mask_sb[:])
                return p2

            def emit_back(ch, p2):
                seq_off, nblk, g, js = ch
                Q0 = seq_off + g * BLK
                key = (seq_off, g)
                if js[0] == g:
                    state[key] = (
                        ps_o.tile([BLK, H, BLK], f32, tag="ot_ps",
                                  name="oT_ps"),
                        ps_sum.tile([BLK, H, BLK], f32, tag="sums_ps",
                                    name="sums_ps"),
                    )
                oT_ps, sums_ps = state[key]
                for jj, j in enumerate(js):
                    if seq_off == 0 and g < nh:
                        vj = v_head[:, j, :]
                    else:
                        vj = v_sb[:, seq_off // BLK + j, :]
                    nc.tensor.matmul(
                        oT_ps[:], vj, p2[:, jj], start=(j == g), stop=(j == 0)
                    )
                for jj, j in enumerate(js):
                    nc.tensor.matmul(
                        sums_ps[:], ones_b[:], p2[:, jj],
                        start=(j == g), stop=(j == 0),
                    )
                if js[-1] == 0:
                    nc.vector.tensor_copy(
                        ot_stage[:, :, Q0 : Q0 + BLK], oT_ps[:]
                    )
                    nc.vector.tensor_copy(
                        sums_stage[:, :, Q0 : Q0 + BLK], sums_ps[0:1]
                    )
                    del state[key]
                    if g == nblk - 1:  # sequence finished: stream outputs
                        Ls = nblk * BLK
                        if seq_off + Ls == T:
                            nc.sync.dma_start(sums_ext[:], sums_stage[:])
                            for g0 in range(0, Ls, BLK):
                                nc.sync.dma_start(
                                    oT_ext[:, :, seq_off + g0 : seq_off + g0 + BLK],
                                    ot_stage[:, :, seq_off + g0 : seq_off + g0 + BLK],
                                )
                        else:
                            nc.sync.dma_start(
                                oT_ext[:, :, seq_off : seq_off + Ls],
                                ot_stage[:, :, seq_off : seq_off + Ls],
                            )

            for ch in chunks:
                p2 = emit_front(ch)
                pending.append((ch, p2))
                if len(pending) > LAG:
                    emit_back(*pending.pop(0))
            for ch, p2 in pending:
                emit_back(ch, p2)

    nc.finalize()
    return nc


def _install_ntff_hook():
    """Shim antenv.axon_hooks (absent in this container) so trace=True can
    reach the terminal's NRT profiler via libaxon_pjrt.so ctypes."""
    import types

    if "antenv.axon_hooks" in sys.modules:
        return
    import antenv
    from concourse import bass_utils

    mod = types.ModuleType("antenv.axon_hooks")
    state = {"hook": None}
    mod.set_axon_ntff_profile_hook = lambda h: state.__setitem__("hook", h)
    mod.get_axon_ntff_profile_hook = lambda: state["hook"]
    sys.modules["antenv.axon_hooks"] = mod
    antenv.axon_hooks = mod
    bass_utils.upload_artifacts = lambda tmpdir: tmpdir  # zero-egress container
    try:
        if "/root/.axon_site" not in sys.path:
            sys.path.insert(0, "/root/.axon_site")
        from trn_agent_boot.trn_boot import _ntff_profile_via_ctypes

        mod.set_axon_ntff_profile_hook(
            _ntff_profile_via_ctypes("/opt/axon/libaxon_pjrt.so")
        )
    except Exception:
        pass


def kernel(q, k, v, cu_seqlens, max_seqlen):
    from concourse import bass_utils

    q = np.asarray(q, dtype=np.float32)
    k = np.asarray(k, dtype=np.float32)
    v = np.asarray(v, dtype=np.float32)
    cu = np.asarray(cu_seqlens, dtype=np.int64)
    T_host = q.shape[0]
    lengths = np.diff(cu).astype(np.int64)
    all_nblocks = [int((L + BLK - 1) // BLK) for L in lengths]
    T_pad = sum(all_nblocks) * BLK

    # process sequences longest-first: big seq warms the pipeline while the
    # rest of the data streams in, and the tail drains a small seq
    order = sorted(range(len(lengths)), key=lambda s: -all_nblocks[s])
    nblocks = [all_nblocks[s] for s in order]

    # host -> padded device token index map (valid tokens only)
    dev_idx = np.zeros(T_host, dtype=np.int64)
    pad_off = 0
    for s in order:
        L = int(lengths[s])
        dev_idx[cu[s] : cu[s] + L] = pad_off + np.arange(L)
        pad_off += all_nblocks[s] * BLK

    bf16 = ml_dtypes.bfloat16
    qp = np.zeros((T_pad, NUM_HEADS * HEAD_DIM), bf16)
    kp = np.zeros((T_pad, NUM_KV_HEADS * HEAD_DIM), bf16)
    vp = np.zeros((T_pad, NUM_KV_HEADS * HEAD_DIM), bf16)
    qp[dev_idx] = q.astype(bf16)
    kp[dev_idx] = k.astype(bf16)
    vp[dev_idx] = v.astype(bf16)

    mask1 = np.where(
        np.arange(BLK)[:, None] <= np.arange(BLK)[None, :], 1.0, 0.0
    ).astype(bf16)
    mask = np.broadcast_to(
        mask1[:, None, :], (BLK, HEADS_PER_CORE, BLK)
    ).copy()

    key = tuple(nblocks)
    if key not in _GRAPH_CACHE:
        _GRAPH_CACHE[key] = _build_graph(key)
    nc = _GRAPH_CACHE[key]

    in_maps = []
    for c in range(N_CORES):
        m = {"mask": mask}
        m["kT"] = np.ascontiguousarray(kp[:, c * HEAD_DIM : (c + 1) * HEAD_DIM].T)
        m["v"] = np.ascontiguousarray(vp[:, c * HEAD_DIM : (c + 1) * HEAD_DIM])
        # [d, h, t] head-interleaved Q^T so all 4 heads ride one 3D AP
        qc = qp[:, c * HEADS_PER_CORE * HEAD_DIM : (c + 1) * HEADS_PER_CORE * HEAD_DIM]
        m["qT"] = np.ascontiguousarray(
            qc.reshape(T_pad, HEADS_PER_CORE, HEAD_DIM).transpose(2, 1, 0)
        )
        in_maps.append(m)

    trace = bool(os.environ.get("BASS_TRACE"))
    if trace:
        _install_ntff_hook()
    res = bass_utils.run_bass_kernel_spmd(
        nc, in_maps, core_ids=list(range(N_CORES)), trace=trace
    )
    if trace and res.exec_time_ns is not None:
        print(f"HW exec time: {res.exec_time_ns} ns")
        if res.instructions_and_trace is not None:
            print(f"trace: {res.instructions_and_trace[1]}")

    out = np.empty((T_host, NUM_HEADS * HEAD_DIM), np.float32)
    for c in range(N_CORES):
        r = res.results[c]
        oT = r["oT"]  # [128, H, T_pad] unnormalized
        sums = r["sums"][0]  # [H, T_pad]
        for h in range(HEADS_PER_CORE):
            gh = c * HEADS_PER_CORE + h
            o = (oT[:, h][:, dev_idx] / sums[h][dev_idx][None, :]).T
            out[:, gh * HEAD_DIM : (gh + 1) * HEAD_DIM] = o
    return out


# revision 68
# speedup vs baseline: 1.0918x; 1.0918x over previous
"""Varlen causal GQA attention on 8 TRN2 NeuronCores.

Problem: 32 q heads, 8 kv heads, head_dim 128, ragged batch (cu_seqlens),
f32. Sharded by KV-head group: core c owns kv head c and q heads
4c..4c+3 — fully data-independent across cores, no collectives.

Per core, blockwise causal attention in 128x128 blocks with all 4 q
heads fused through 3D access patterns (q stored head-interleaved
[d, h, t]), so every matmul streams exactly 4*128 = 512 columns and
each PSUM bank carries exactly one accumulation chain:
    S[k, h, q]  = (K_j)^T.T @ Q^T      ONE matmul per (q-block, k-block)
    P = exp(S * scale)                 ONE ScalarE op
    causal mask: 0/1 multiply on GpSimd (diagonal blocks only)
    O^T[h] += V_j @ P                  ONE matmul, PSUM-accumulated over j
    sums[h] += ones.T @ P              ONE M=1 matmul
Host does all transposes (Q^T/K^T in, O^T -> O out), bf16 conversion,
and the final softmax division.
"""

import math
import os
import sys

sys.path.insert(0, "/opt/trn_rl_repo")

import ml_dtypes
import numpy as np

NUM_HEADS = 32
NUM_KV_HEADS = 8
HEAD_DIM = 128
HEADS_PER_CORE = NUM_HEADS // NUM_KV_HEADS  # 4
N_CORES = 8
BLK = 128
SCALE = 1.0 / math.sqrt(HEAD_DIM)

_GRAPH_CACHE = {}


def _build_graph(seq_blocks):
    """Build the SPMD Bacc graph for padded per-seq block counts."""
    from concourse import bacc
    import concourse.mybir as mybir
    from concourse.tile import TileContext

    f32 = mybir.dt.float32
    bf16 = mybir.dt.bfloat16
    T = sum(seq_blocks) * BLK
    n_blocks_total = T // BLK
    H = HEADS_PER_CORE

    nc = bacc.Bacc("TRN2", target_bir_lowering=False, debug=False,
                   num_devices=N_CORES)

    qT_ext = nc.declare_dram_parameter("qT", [BLK, H, T], bf16, isOutput=False)
    kT_ext = nc.declare_dram_parameter("kT", [BLK, T], bf16, isOutput=False)
    v_ext = nc.declare_dram_parameter("v", [T, HEAD_DIM], bf16, isOutput=False)
    mask_ext = nc.declare_dram_parameter("mask", [BLK, H, BLK], bf16,
                                         isOutput=False)
    oT_ext = nc.declare_dram_parameter("oT", [BLK, H, T], f32, isOutput=True)
    sums_ext = nc.declare_dram_parameter("sums", [1, H, T], f32, isOutput=True)

    with TileContext(nc) as tc:
        with (
            tc.tile_pool(name="persist", bufs=1) as persist,
            tc.tile_pool(name="p", bufs=8) as p_pool,
            tc.tile_pool(name="ps_s4", bufs=2, space="PSUM") as ps_s4,
            tc.tile_pool(name="ps_o", bufs=2, space="PSUM") as ps_o,
            tc.tile_pool(name="ps_sum", bufs=2, space="PSUM") as ps_sum,
        ):
            kT_sb = persist.tile([BLK, T], bf16)
            v_sb = persist.tile([BLK, n_blocks_total, HEAD_DIM], bf16)
            mask_sb = persist.tile([BLK, H, BLK], bf16)
            qT_sb = persist.tile([BLK, H, T], bf16)
            v_re = v_ext[:].rearrange("(j p) d -> p j d", p=BLK)
            nb0 = seq_blocks[0]
            # dependencies are tile-granular: duplicate the first few blocks
            # into separate small tiles so the opening matmuls depend only on
            # tiny DMAs, not on the full-tensor loads running behind them
            nh = min(6, nb0)
            c00 = nh * BLK
            kT_head = persist.tile([BLK, c00], bf16)
            qT_head = persist.tile([BLK, H, c00], bf16)
            v_head = persist.tile([BLK, nh, HEAD_DIM], bf16)
            # one ring, strict order: small head tiles land first (the ring
            # serializes transfers), then seq-0 bulk; other seqs + mask go on
            # the Scalar ring where they can't delay the heads
            nc.sync.dma_start(qT_head[:], qT_ext[:, :, :c00])
            nc.sync.dma_start(kT_head[:], kT_ext[:, :c00])
            nc.sync.dma_start(v_head[:], v_re[:, :nh, :])
            nc.sync.dma_start(qT_sb[:, :, : nb0 * BLK],
                              qT_ext[:, :, : nb0 * BLK])
            nc.sync.dma_start(kT_sb[:, : nb0 * BLK], kT_ext[:, : nb0 * BLK])
            nc.sync.dma_start(v_sb[:, :nb0, :], v_re[:, :nb0, :])
            nc.scalar.dma_start(mask_sb[:], mask_ext[:])
            if nb0 < n_blocks_total:
                c0 = nb0 * BLK
                nc.scalar.dma_start(kT_sb[:, c0:], kT_ext[:, c0:])
                nc.scalar.dma_start(v_sb[:, nb0:, :], v_re[:, nb0:, :])
                nc.scalar.dma_start(qT_sb[:, :, c0:], qT_ext[:, :, c0:])

            ones_f = persist.tile([BLK, BLK], f32)
            nc.vector.memset(ones_f[:], 1.0)
            # full [128,128] ones stationary: sums matmul runs M=128 so the
            # PE array never reconfigures col groups between AV and sums
            ones_b = persist.tile([BLK, BLK], bf16)
            nc.vector.tensor_copy(ones_b[:], ones_f[:])

            ot_stage = persist.tile([BLK, H, T], f32)
            sums_stage = persist.tile([1, H, T], f32)

            # flat chunk stream over (seq, q-block g, k-block j-pairs), with
            # j descending inside each group (diagonal/masked block first).
            # Each chunk = up to 2 consecutive j's whose S tiles share one
            # 2-bank PSUM tile [128, jj, h, q] so ONE exp covers both.
            chunks = []
            seq_off = 0
            for nblk in seq_blocks:
                for g in range(nblk):
                    js = list(range(g, -1, -1))
                    for i0 in range(0, len(js), 2):
                        chunks.append((seq_off, nblk, g, js[i0 : i0 + 2]))
                seq_off += nblk * BLK

            # software-pipelined ISSUE order: AV/sums trail their S/exp by
            # LAG chunks so the PE's FIFO never parks on an exp wait
            LAG = 3
            state = {}  # live group accumulators keyed by (seq_off, g)
            pending = []

            def emit_front(ch):
                seq_off, nblk, g, js = ch
                Q0 = seq_off + g * BLK
                s2 = ps_s4.tile([BLK, 2, H, BLK], f32, tag="s2", name="s2")
                for jj, j in enumerate(js):
                    if seq_off == 0 and g < nh:
                        kj = kT_head[:, j * BLK : (j + 1) * BLK]
                        qg = qT_head[:, :, g * BLK : (g + 1) * BLK]
                    else:
                        kj = kT_sb[:, seq_off + j * BLK : seq_off + (j + 1) * BLK]
                        qg = qT_sb[:, :, Q0 : Q0 + BLK]
                    nc.tensor.matmul(
                        s2[:, jj], kj, qg,
                        start=True, stop=True,
                    )
                p2 = p_pool.tile([BLK, 2, H, BLK], bf16, tag="p2", name="p2")
                nj = len(js)
                nc.scalar.activation(
                    p2[:, :nj], s2[:, :nj],
                    mybir.ActivationFunctionType.Exp,
                    scale=SCALE,
                )
                if js[0] == g:  # diagonal: zero the upper triangle
                    nc.gpsimd.tensor_mul(p2[:, 0], p2[:, 0], mask_sb[:])
                return p2

            def emit_back(ch, p2):
                seq_off, nblk, g, js = ch
                Q0 = seq_off + g * BLK
                key = (seq_off, g)
                if js[0] == g:
                    state[key] = (
                        ps_o.tile([BLK, H, BLK], f32, tag="ot_ps",
                                  name="oT_ps"),
                        ps_sum.tile([BLK, H, BLK], f32, tag="sums_ps",
                                    name="sums_ps"),
                    )
                oT_ps, sums_ps = state[key]
                for jj, j in enumerate(js):
                    if seq_off == 0 and g < nh:
                        vj = v_head[:, j, :]
                    else:
                        vj = v_sb[:, seq_off // BLK + j, :]
                    nc.tensor.matmul(
                        oT_ps[:], vj, p2[:, jj], start=(j == g), stop=(j == 0)
                    )
                for jj, j in enumerate(js):
                    nc.tensor.matmul(
                        sums_ps[:], ones_b[:], p2[:, jj],
                        start=(j == g), stop=(j == 0),
                    )
                if js[-1] == 0:
                    nc.vector.tensor_copy(
                        ot_stage[:, :, Q0 : Q0 + BLK], oT_ps[:]
                    )
                    nc.vector.tensor_copy(
                        sums_stage[:, :, Q0 : Q0 + BLK], sums_ps[0:1]
                    )
                    del state[key]
                    if g == nblk - 1:  # sequence finished: stream outputs
                        Ls = nblk * BLK
                        if seq_off + Ls == T:
                            nc.sync.dma_start(sums_ext[:], sums_stage[:])
                            for g0 in range(0, Ls, BLK):
                                nc.sync.dma_start(
                                    oT_ext[:, :, seq_off + g0 : seq_off + g0 + BLK],
                                    ot_stage[:, :, seq_off + g0 : seq_off + g0 + BLK],
                                )
                        else:
                            nc.sync.dma_start(
                                oT_ext[:, :, seq_off : seq_off + Ls],
                                ot_stage[:, :, seq_off : seq_off + Ls],
                            )

            for ch in chunks:
                p2 = emit_front(ch)
                pending.append((ch, p2))
                if len(pending) > LAG:
                    emit_back(*pending.pop(0))
            for ch, p2 in pending:
                emit_back(ch, p2)

    nc.finalize()
    return nc


def _install_ntff_hook():
    """Shim antenv.axon_hooks (absent in this container) so trace=True can
    reach the terminal's NRT profiler via libaxon_pjrt.so ctypes."""
    import types

    if "antenv.axon_hooks" in sys.modules:
        return
    import antenv
    from concourse import bass_utils

    mod = types.ModuleType("antenv.axon_hooks")
    state = {"hook": None}
    mod.set_axon_ntff_profile_hook = lambda h: state.__setitem__("hook", h)
    mod.get_axon_ntff_profile_hook = lambda: state["hook"]
    sys.modules["antenv.axon_hooks"] = mod
    antenv.axon_hooks = mod
    bass_utils.upload_artifacts = lambda tmpdir: tmpdir  # zero-egress container
    try:
        if "/root/.axon_site" not in sys.path:
            sys.path.insert(0, "/root/.axon_site")
        from trn_agent_boot.trn_boot import _ntff_profile_via_ctypes

        mod.set_axon_ntff_profile_hook(
            _ntff_profile_via_ctypes("/opt/axon/libaxon_pjrt.so")
        )
    except Exception:
        pass


def kernel(q, k, v, cu_seqlens, max_seqlen):
    from concourse import bass_utils

    q = np.asarray(q, dtype=np.float32)
    k = np.asarray(k, dtype=np.float32)
    v = np.asarray(v, dtype=np.float32)
    cu = np.asarray(cu_seqlens, dtype=np.int64)
    T_host = q.shape[0]
    lengths = np.diff(cu).astype(np.int64)
    all_nblocks = [int((L + BLK - 1) // BLK) for L in lengths]
    T_pad = sum(all_nblocks) * BLK

    # process sequences longest-first: big seq warms the pipeline while the
    # rest of the data streams in, and the tail drains a small seq
    order = sorted(range(len(lengths)), key=lambda s: -all_nblocks[s])
    nblocks = [all_nblocks[s] for s in order]

    # host -> padded device token index map (valid tokens only)
    dev_idx = np.zeros(T_host, dtype=np.int64)
    pad_off = 0
    for s in order:
        L = int(lengths[s])
        dev_idx[cu[s] : cu[s] + L] = pad_off + np.arange(L)
        pad_off += all_nblocks[s] * BLK

    bf16 = ml_dtypes.bfloat16
    qp = np.zeros((T_pad, NUM_HEADS * HEAD_DIM), bf16)
    kp = np.zeros((T_pad, NUM_KV_HEADS * HEAD_DIM), bf16)
    vp = np.zeros((T_pad, NUM_KV_HEADS * HEAD_DIM), bf16)
    qp[dev_idx] = q.astype(bf16)
    kp[dev_idx] = k.astype(bf16)
    vp[dev_idx] = v.astype(bf16)

    mask1 = np.where(
        np.arange(BLK)[:, None] <= np.arange(BLK)[None, :], 1.0, 0.0
    ).astype(bf16)
    mask = np.broadcast_to(
        mask1[:, None, :], (BLK, HEADS_PER_CORE, BLK)
    ).copy()

    key = tuple(nblocks)
    if key not in _GRAPH_CACHE:
        _GRAPH_CACHE[key] = _build_graph(key)
    nc = _GRAPH_CACHE[key]

    in_maps = []
    for c in range(N_CORES):
        m = {"mask": mask}
        m["kT"] = np.ascontiguousarray(kp[:, c * HEAD_DIM : (c + 1) * HEAD_DIM].T)
        m["v"] = np.ascontiguousarray(vp[:, c * HEAD_DIM : (c + 1) * HEAD_DIM])
        # [d, h, t] head-interleaved Q^T so all 4 heads ride one 3D AP
        qc = qp[:, c * HEADS_PER_CORE * HEAD_DIM : (c + 1) * HEADS_PER_CORE * HEAD_DIM]
        m["qT"] = np.ascontiguousarray(
            qc.reshape(T_pad, HEADS_PER_CORE, HEAD_DIM).transpose(2, 1, 0)
        )
        in_maps.append(m)

    trace = bool(os.environ.get("BASS_TRACE"))
    if trace:
        _install_ntff_hook()
    res = bass_utils.run_bass_kernel_spmd(
        nc, in_maps, core_ids=list(range(N_CORES)), trace=trace
    )
    if trace and res.exec_time_ns is not None:
        print(f"HW exec time: {res.exec_time_ns} ns")
        if res.instructions_and_trace is not None:
            print(f"trace: {res.instructions_and_trace[1]}")

    out = np.empty((T_host, NUM_HEADS * HEAD_DIM), np.float32)
    for c in range(N_CORES):
        r = res.results[c]
        oT = r["oT"]  # [128, H, T_pad] unnormalized
        sums = r["sums"][0]  # [H, T_pad]
        for h in range(HEADS_PER_CORE):
            gh = c * HEADS_PER_CORE + h
            o = (oT[:, h][:, dev_idx] / sums[h][dev_idx][None, :]).T
            out[:, gh * HEAD_DIM : (gh + 1) * HEAD_DIM] = o
    return out
